# revision 21
# baseline (speedup 1.0000x reference)
"""Multi-head causal attention on 8 Trainium2 NeuronCores (Bass/Tile).

Sharding: batch x head-group. Core c handles batch c//4 and the 4 heads
[(c%4)*4, (c%4)*4+4). Each core computes a partial output projection
[S, D] for its heads; the host sums the 4 partials per batch and adds b_O.

Per-core kernel (all matmuls in float32r = full-rate fp32):
  - host supplies x^T (plus a ones row for bias folding when biases != 0)
  - Q^T,K^T computed head-pair-packed [128, S]; V in natural [s, dh] layout
    with an appended ones column (yields the softmax denominator as row 64
    of the AV product)
  - scores computed as S^T = K^T.T @ Q^T  ->  [k, q] tiles, exp on ACT with
    fused 1/sqrt(dh) scale (no max subtraction: |scores*scale| is small for
    this operand distribution, exp is safe in fp32 and matches softmax
    exactly up to rounding)
  - causal mask: fully-masked 128-col spans are memset to zero (exp is only
    computed on the live span), the diagonal 128x128 band is multiplied by
    a 0/1 triangular mask on GPSIMD
  - AV: Z'^T[65, q] accumulated over k tiles in PSUM; the k-loop is
    software-pipelined (scores/exp run SKEW tiles ahead of AV)
  - normalize: denominator row -> SBUF via DVE, reciprocal on DVE,
    broadcast across partitions with a K=1 PE outer product, multiply on DVE
  - out projection: lhsT = Z^T pair-packed [128(dh of 2 heads), q],
    accumulating both head pairs into one PSUM tile, bounce SBUF, DMA out
"""

import numpy as np

B, S, D, DH, H = 2, 2048, 1024, 64, 16
HLOC = 4  # heads per core
P = 128
QG = 512  # q-group width
NQG = S // QG  # 4
NKT = S // P  # 16
NDC = D // P  # 8
SCALE = 1.0 / float(np.sqrt(DH))
SKEW = 3  # scores/exp tiles in flight ahead of AV

MM_DTYPE = "float32r"  # "float32r" | "float32" | "bfloat16"

_CACHE = {}
LAST_RESULT = None
TRACE = False


def _build_program(mmdt_name, with_bias, repeats=1):
    from contextlib import ExitStack

    import concourse.bacc as bacc
    import concourse.mybir as mybir
    import concourse.tile as tile

    F32 = mybir.dt.float32
    MMDT = getattr(mybir.dt, mmdt_name)
    Exp = mybir.ActivationFunctionType.Exp

    nc = bacc.Bacc("TRN2", target_bir_lowering=False, debug=False, num_devices=8)
    xt_d = nc.dram_tensor("xt", [D + 1, S], MMDT, kind="ExternalInput").ap()
    wq_d = nc.dram_tensor("wq", [D + 1, 256], MMDT, kind="ExternalInput").ap()
    wk_d = nc.dram_tensor("wk", [D + 1, 256], MMDT, kind="ExternalInput").ap()
    wv_d = nc.dram_tensor("wv", [D + 1, 256], MMDT, kind="ExternalInput").ap()
    wo_d = nc.dram_tensor("wo", [256, D], MMDT, kind="ExternalInput").ap()
    m01_d = nc.dram_tensor("m01", [P, P], MMDT, kind="ExternalInput").ap()
    out_d = nc.dram_tensor("out", [S, D], F32, kind="ExternalOutput").ap()

    def r(ap):
        return ap

    with tile.TileContext(nc) as tc, ExitStack() as ctx:
        wpool = ctx.enter_context(tc.tile_pool(name="wpool", bufs=1))
        spool = ctx.enter_context(tc.tile_pool(name="spool", bufs=1))
        xpool = ctx.enter_context(tc.tile_pool(name="xpool", bufs=16))
        ptpool = ctx.enter_context(tc.tile_pool(name="ptpool", bufs=6))
        npool = ctx.enter_context(tc.tile_pool(name="npool", bufs=4))
        mmp = ctx.enter_context(tc.tile_pool(name="mmp", bufs=3, space="PSUM"))
        zpp = ctx.enter_context(tc.tile_pool(name="zpp", bufs=2, space="PSUM"))
        drp = ctx.enter_context(tc.tile_pool(name="drp", bufs=4, space="DRAM"))

        # ---- weights / masks / constants ----
        wq_t, wk_t, wv_t = [], [], []
        for nm, src, lst in (("wqt", wq_d, wq_t), ("wkt", wk_d, wk_t), ("wvt", wv_d, wv_t)):
            for cch in range(NDC):
                t = wpool.tile([P, 256], MMDT, name=f"{nm}{cch}")
                nc.gpsimd.dma_start(out=t, in_=src[cch * P:(cch + 1) * P, :])
                lst.append(t)
        if with_bias:
            wqb = wpool.tile([1, 256], MMDT, name="wqb")
            nc.gpsimd.dma_start(out=wqb, in_=wq_d[D:D + 1, :])
            wkb = wpool.tile([1, 256], MMDT, name="wkb")
            nc.gpsimd.dma_start(out=wkb, in_=wk_d[D:D + 1, :])
            wvb = wpool.tile([1, 256], MMDT, name="wvb")
            nc.gpsimd.dma_start(out=wvb, in_=wv_d[D:D + 1, :])
        wo_t = []
        for pr in range(2):
            t = wpool.tile([P, D], MMDT, name=f"wot{pr}")
            nc.gpsimd.dma_start(out=t, in_=wo_d[pr * P:(pr + 1) * P, :])
            wo_t.append(t)
        tri = wpool.tile([P, P], MMDT, name="tri")
        nc.gpsimd.dma_start(out=tri, in_=m01_d)
        zbias = wpool.tile([P, 1], F32, name="zbias")
        nc.vector.memset(zbias, 0.0)

        # ---- persistent activations ----
        QT = [spool.tile([P, S], MMDT, name=f"qt{pr}") for pr in range(2)]
        KT = [spool.tile([P, S], MMDT, name=f"kt{pr}") for pr in range(2)]
        VP = spool.tile([P, NKT * HLOC * 65], MMDT, name="vpk")  # [128, 4160]
        ZT = [spool.tile([P, S], MMDT, name=f"zt{pr}") for pr in range(2)]
        # whole-tile memset to 1.0; the V scatter copies overwrite all but
        # the per-head ones columns (f32 bitcast: memset on f32r is invalid ISA)
        nc.vector.memset(VP.bitcast(F32), 1.0)

        # ---- interleaved schedule ----
        # For each 512-wide chunk g: project chunk g (Q^T/K^T cols, V k-tiles),
        # run attention group g for all 4 heads (only needs chunks <= g),
        # then the output projection for group g's q-tiles. Normalize
        # sequences are deferred into the next head's k-loop so the PE
        # never stalls on the reciprocal chain.
        pending_norm = []

        def flush_norm():
            while pending_norm:
                pending_norm.pop(0)()

        def phase1_chunk(sc):
            xts = []
            for cch in range(NDC):
                t = xpool.tile([P, QG], MMDT, name="xts", tag="xts", bufs=12)
                nc.sync.dma_start(out=t, in_=xt_d[cch * P:(cch + 1) * P, sc * QG:(sc + 1) * QG])
                xts.append(t)
            xon = None
            if with_bias:
                xon = xpool.tile([1, QG], MMDT, name="xon", tag="xon", bufs=2)
                nc.sync.dma_start(out=xon, in_=xt_d[D:D + 1, sc * QG:(sc + 1) * QG])
            for pr in range(2):
                for wt, wb, dstl in (
                    (wq_t, wqb if with_bias else None, QT),
                    (wk_t, wkb if with_bias else None, KT),
                ):
                    pp = mmp.tile([P, QG], F32, name="pp", tag="mm")
                    for cch in range(NDC):
                        nc.tensor.matmul(
                            pp, lhsT=r(wt[cch][:, pr * 128:(pr + 1) * 128]),
                            rhs=r(xts[cch]), start=(cch == 0),
                            stop=(not with_bias and cch == NDC - 1))
                    if with_bias:
                        nc.tensor.matmul(
                            pp, lhsT=r(wb[:, pr * 128:(pr + 1) * 128]), rhs=r(xon),
                            start=False, stop=True)
                    nc.vector.tensor_copy(dstl[pr][:, sc * QG:(sc + 1) * QG], pp)
            for st in range(4):
                kt = sc * 4 + st
                vv = mmp.tile([P, 256], F32, name="vv", tag="mm")
                for cch in range(NDC):
                    nc.tensor.matmul(
                        vv, lhsT=r(xts[cch][:, st * P:(st + 1) * P]), rhs=r(wv_t[cch]),
                        start=(cch == 0), stop=(not with_bias and cch == NDC - 1))
                if with_bias:
                    nc.tensor.matmul(
                        vv, lhsT=r(xon[:, st * P:(st + 1) * P]), rhs=r(wvb),
                        start=False, stop=True)
                dst = VP[:, kt * 260:(kt + 1) * 260].rearrange("p (h c) -> p h c", h=HLOC)[:, :, 0:64]
                nc.vector.tensor_copy(dst, vv.rearrange("p (h c) -> p h c", h=HLOC))

        def attention_head_group(h, g):
            pr, hf = h // 2, h % 2
            QTh = QT[pr][hf * 64:(hf + 1) * 64, :]
            KTh = KT[pr][hf * 64:(hf + 1) * 64, :]
            nkt = 4 * g + 4
            zp = zpp.tile([P, QG], F32, name="zp", tag="zp")
            pts = {}

            # live-span start (in columns) for matmuls of diagonal tile j:
            # everything left of 128j is causally dead; keep N >= 256 so
            # float32r stays at full rate
            LO = {0: 0, 1: P, 2: 2 * P, 3: 2 * P}

            def make_pt(kt):
                j = kt - 4 * g
                lo = LO[j] if j >= 0 else 0
                sp = mmp.tile([P, QG], F32, name="sp", tag="sp", bufs=3)
                nc.tensor.matmul(
                    sp[:, lo:], lhsT=r(KTh[:, kt * P:(kt + 1) * P]),
                    rhs=r(QTh[:, g * QG + lo:(g + 1) * QG]), start=True, stop=True)
                pt = ptpool.tile([P, QG], MMDT, name="pt", tag="pt", bufs=8)
                if j <= 0:
                    nc.scalar.activation(pt, sp, Exp, bias=zbias, scale=SCALE)
                else:
                    # exp only on the causally live span; zero the dead gap
                    # that the AV matmul will still read
                    if j * P > lo:
                        nc.vector.memset(pt[:, lo:j * P].bitcast(F32), 0.0)
                    nc.scalar.activation(
                        pt[:, j * P:], sp[:, j * P:], Exp, bias=zbias, scale=SCALE)
                if j >= 0:
                    # triangular band on the diagonal 128x128 block
                    nc.gpsimd.tensor_mul(
                        pt[:, j * P:(j + 1) * P], pt[:, j * P:(j + 1) * P], tri)
                pts[kt] = (pt, lo)

            def do_av(kt):
                pt, lo = pts.pop(kt)
                nc.tensor.matmul(
                    zp[0:65, lo:],
                    lhsT=r(VP[:, kt * 260 + h * 65: kt * 260 + (h + 1) * 65]),
                    rhs=r(pt[:, lo:]), start=(kt == 0), stop=(kt == nkt - 1))

            for step in range(nkt + SKEW):
                if step < nkt:
                    make_pt(step)
                if step == SKEW - 1:
                    flush_norm()
                if step >= SKEW:
                    do_av(step - SKEW)

            def normalize():
                rec = npool.tile([1, QG], F32, name="rec", tag="rec", bufs=2)
                nc.vector.reciprocal(rec, zp[64:65, :])
                recd = drp.tile([1, QG], F32, name="recd", tag="recd")
                nc.sync.dma_start(out=recd, in_=rec)
                rb = npool.tile([64, QG], F32, name="rb", tag="rb", bufs=2)
                nc.sync.dma_start(out=rb, in_=recd[0, :].partition_broadcast(64))
                nc.vector.tensor_mul(
                    ZT[pr][hf * 64:(hf + 1) * 64, g * QG:(g + 1) * QG],
                    zp[0:64, :], rb)

            pending_norm.append(normalize)

        def outproj_group(g):
            for qt in range(4 * g, 4 * g + 4):
                for chk in range(2):
                    op = mmp.tile([P, QG], F32, name="op", tag="mm")
                    for pr in range(2):
                        nc.tensor.matmul(
                            op, lhsT=r(ZT[pr][:, qt * P:(qt + 1) * P]),
                            rhs=r(wo_t[pr][:, chk * QG:(chk + 1) * QG]),
                            start=(pr == 0), stop=(pr == 1))
                    ob = ptpool.tile([P, QG], F32, name="ob", tag="ob", bufs=3)
                    nc.vector.tensor_copy(ob, op)
                    nc.scalar.dma_start(
                        out=out_d[qt * P:(qt + 1) * P, chk * QG:(chk + 1) * QG], in_=ob)

        def whole_body():
            phase1_chunk(0)
            for g in range(NQG):
                for h in range(HLOC):
                    attention_head_group(h, g)
                if g + 1 < NQG:
                    phase1_chunk(g + 1)
                flush_norm()
                outproj_group(g)

        if repeats == 1:
            whole_body()
        else:
            with tc.For_i(0, repeats, 1):
                whole_body()

    nc.compile()
    return nc


BENCH_REPEATS = 1


def _get_program(with_bias=True):
    key = (MM_DTYPE, with_bias, BENCH_REPEATS)
    if key not in _CACHE:
        _CACHE[key] = _build_program(MM_DTYPE, with_bias, BENCH_REPEATS)
    return _CACHE[key]


def _tri_mask():
    qq = np.arange(P)[None, :]
    pp = np.arange(P)[:, None]
    return (qq >= pp).astype(np.float32)


def _patch_walrus_errors():
    # surface walrus compile errors (the PJRT custom-call hook swallows them)
    import subprocess

    import concourse.bass_utils as bu

    if getattr(bu, "_werr_patched", False):
        return
    orig_run = bu.run_command

    def run2(argv, **kw):
        try:
            return orig_run(argv, **kw)
        except subprocess.CalledProcessError as e:
            import sys
            print("==== WALRUS COMPILE FAILURE ====", file=sys.stderr)
            if e.output:
                print(e.output[-6000:], file=sys.stderr)
            raise

    bu.run_command = run2
    bu._werr_patched = True


_RUNNERS = {}


def _build_runner(with_bias, repeats):
    """Build the bass program + jitted shard_map executable once; reuse across
    calls. Mirrors concourse.bass2jax.run_bass_via_pjrt exactly (numpy inputs,
    donated zero output buffers) but caches the traced jit."""
    import jax
    from jax.sharding import Mesh, PartitionSpec
    from jax.experimental.shard_map import shard_map

    import concourse.mybir as mybir
    from concourse.bass2jax import (
        _bass_exec_p,
        install_neuronx_cc_hook,
        partition_id_tensor,
    )

    _patch_walrus_errors()
    install_neuronx_cc_hook()
    nc = _get_program(with_bias)

    partition_name = nc.partition_id_tensor.name if nc.partition_id_tensor else None
    in_names, out_names, out_avals, zero_shapes = [], [], [], []
    for alloc in nc.m.functions[0].allocations:
        if not isinstance(alloc, mybir.MemoryLocationSet):
            continue
        name = alloc.memorylocations[0].name
        if alloc.kind == "ExternalInput":
            if name != partition_name:
                in_names.append(name)
        elif alloc.kind == "ExternalOutput":
            out_names.append(name)
            shape = tuple(alloc.tensor_shape)
            dtype = mybir.dt.np(alloc.dtype)
            out_avals.append(jax.core.ShapedArray(shape, dtype))
            zero_shapes.append((shape, dtype))
    all_names = in_names + out_names + ([partition_name] if partition_name else [])
    nin = len(in_names)

    def _body(*args):
        operands = list(args)
        if partition_name is not None:
            operands.append(partition_id_tensor())
        return tuple(_bass_exec_p.bind(
            *operands, out_avals=tuple(out_avals), in_names=tuple(all_names),
            out_names=tuple(out_names), lowering_input_output_aliases=(),
            sim_require_finite=True, sim_require_nnan=True, nc=nc))

    devices = jax.devices()[:8]
    mesh = Mesh(np.asarray(devices), ("core",))
    nout = len(out_names)
    bass_fn = jax.jit(
        shard_map(
            _body, mesh=mesh,
            in_specs=(PartitionSpec("core"),) * (nin + nout),
            out_specs=(PartitionSpec("core"),) * nout, check_rep=False),
        donate_argnums=tuple(range(nin, nin + nout)), keep_unused=True)

    def run(in_maps):
        per_core = [[np.asarray(m[name]) for name in in_names] for m in in_maps]
        concat_in = [
            np.concatenate([per_core[c][i] for c in range(8)], axis=0)
            for i in range(nin)
        ]
        zeros = [np.zeros((8 * s[0], *s[1:]), d) for s, d in zero_shapes]
        outs = bass_fn(*concat_in, *zeros)
        return np.asarray(outs[0])

    return run


def kernel(**inputs):
    x = np.asarray(inputs["normalized_resid_pre"], dtype=np.float32)
    W_Q = np.asarray(inputs["W_Q"], dtype=np.float32)
    W_K = np.asarray(inputs["W_K"], dtype=np.float32)
    W_V = np.asarray(inputs["W_V"], dtype=np.float32)
    W_O = np.asarray(inputs["W_O"], dtype=np.float32)
    b_Q = np.asarray(inputs["b_Q"], dtype=np.float32)
    b_K = np.asarray(inputs["b_K"], dtype=np.float32)
    b_V = np.asarray(inputs["b_V"], dtype=np.float32)
    b_O = np.asarray(inputs["b_O"], dtype=np.float32)

    with_bias = bool(np.any(b_Q) or np.any(b_K) or np.any(b_V))
    key = (MM_DTYPE, with_bias, BENCH_REPEATS)
    if key not in _RUNNERS:
        _RUNNERS[key] = _build_runner(with_bias, BENCH_REPEATS)

    tri = _tri_mask()
    ones_row = np.ones((1, S), dtype=np.float32)
    xt = [np.ascontiguousarray(
        np.concatenate([x[b].T, ones_row], axis=0)) for b in range(B)]

    def pack_w(Wh, bh):  # Wh [4, 1024, 64], bh [4, 64] -> [1025, 256]
        w = np.concatenate([Wh[k] for k in range(HLOC)], axis=1)
        bias = np.concatenate([bh[k] for k in range(HLOC)])[None, :]
        return np.ascontiguousarray(np.concatenate([w, bias], axis=0))

    in_maps = []
    for c in range(8):
        b, hg = c // 4, c % 4
        hs = hg * HLOC
        in_maps.append({
            "xt": xt[b],
            "wq": pack_w(W_Q[hs:hs + HLOC], b_Q[hs:hs + HLOC]),
            "wk": pack_w(W_K[hs:hs + HLOC], b_K[hs:hs + HLOC]),
            "wv": pack_w(W_V[hs:hs + HLOC], b_V[hs:hs + HLOC]),
            "wo": np.ascontiguousarray(
                np.concatenate([W_O[hs + k] for k in range(HLOC)], axis=0)),
            "m01": tri,
        })

    out_g = _RUNNERS[key](in_maps)
    parts = out_g.reshape(8, S, D)
    out0 = parts[0:4].sum(axis=0) + b_O[None, :]
    out1 = parts[4:8].sum(axis=0) + b_O[None, :]
    return np.stack([out0, out1]).astype(np.float32)
